# revision 55
# baseline (speedup 1.0000x reference)
"""Trainium2 Bass kernel for nn_Attention_24043226923261.

Per-pixel cross-attention: RMSNorm(c) -> kv proj -> softmax over N=8 context
slices with a query shared across the 32x32 spatial grid -> out proj.

Sharding: data-parallel over B=8 across the 8 NeuronCores (core b owns batch
b). Zero collectives.

Key algebraic restructuring (host-side weight folding, exact math):
  - query path qh = silu(emb[q]@w1+b1)@w2+b2 is a [8,512] tensor; dots =
    qh . (c_norm @ w_k) = c_norm @ (w_k @ qh^T), so fold qh, attn_scale and
    rms_w into a per-core [256,8] matrix wq.  k is never materialized and the
    kv projection halves to v-only.
  - rms_w folds into wv/wq; the per-token rsqrt(mean(c^2)) scale s_n[t] is
    applied on device: on the k side inside exp() via the activation's
    per-partition scale, on the v side by folding into the softmax weights.
  - out proj is computed transposed (out^T = w_out^T @ h^T) so the result
    lands channel-major [256, H*W], which is exactly the required output
    layout.
"""

import sys

for _p in ("/opt/trn_rl_repo",):
    if _p not in sys.path:
        sys.path.insert(0, _p)

import numpy as np


B = 8
N = 8          # context slices (softmax axis)
CH = 256       # channels / hidden
H = W = 32
T = H * W      # 1024 spatial tokens per batch
HEADS = 8
HD = 64        # head dim
HS = HEADS * HD  # 512
EPS = 1e-6
NCORES = 8
PT = 128       # partition tile
TT = T // PT   # 8 token tiles
KCH = CH // PT  # 2 contraction chunks over channels
KHS = HS // PT  # 4 contraction chunks over (head, d)
GRP = 4        # token tiles per out-proj batch


def _kernel_body(nc, tc, d):
    from contextlib import ExitStack

    from concourse import mybir

    AF = mybir.ActivationFunctionType
    ALU = mybir.AluOpType
    AX = mybir.AxisListType
    f32 = mybir.dt.float32
    bf16 = mybir.dt.bfloat16

    with ExitStack() as ctx:
        const = ctx.enter_context(tc.tile_pool(name="const", bufs=1))
        cpool = ctx.enter_context(tc.tile_pool(name="c", bufs=1))
        c2p = ctx.enter_context(tc.tile_pool(name="c2", bufs=5))
        sp = ctx.enter_context(tc.tile_pool(name="s", bufs=1))
        ep = ctx.enter_context(tc.tile_pool(name="e", bufs=2))
        avp = ctx.enter_context(tc.tile_pool(name="av", bufs=3))
        hp = ctx.enter_context(tc.tile_pool(name="h", bufs=2))
        prodp = ctx.enter_context(tc.tile_pool(name="prod", bufs=5))
        htp = ctx.enter_context(tc.tile_pool(name="ht", bufs=2))
        outp = ctx.enter_context(tc.tile_pool(name="o", bufs=2))
        psD = ctx.enter_context(tc.tile_pool(name="psD", bufs=1, space="PSUM"))
        psV = ctx.enter_context(tc.tile_pool(name="psV", bufs=3, space="PSUM"))
        psT = ctx.enter_context(tc.tile_pool(name="psT", bufs=2, space="PSUM"))
        psO = ctx.enter_context(tc.tile_pool(name="psO", bufs=2, space="PSUM"))

        # ---- constants + c loads, ordered so c[0] lands early ----
        wq_sb = []
        invc_sb = []
        for k in range(KCH):
            t = const.tile([PT, HEADS], bf16, tag=f"wq{k}", name=f"wq{k}")
            nc.sync.dma_start(t[:], d["wq"][k * PT:(k + 1) * PT, :])
            wq_sb.append(t)
            t = const.tile([PT, 1], bf16, tag=f"invc{k}", name=f"invc{k}")
            nc.sync.dma_start(t[:], d["invc"][k * PT:(k + 1) * PT, :])
            invc_sb.append(t)
        eps_sb = const.tile([PT, 1], f32, tag="eps", name="eps")
        nc.vector.memset(eps_sb[:], EPS)

        c_sb = {}
        for k in range(KCH):
            t = cpool.tile([PT, T], bf16, tag=f"c0_{k}", name=f"c0_{k}")
            nc.sync.dma_start(t[:], d["c"][0, k * PT:(k + 1) * PT, :])
            c_sb[0, k] = t

        wv_sb = []
        for k in range(KCH):
            t = const.tile([PT, HS], bf16, tag=f"wv{k}", name=f"wv{k}")
            nc.sync.dma_start(t[:], d["wv"][k * PT:(k + 1) * PT, :])
            wv_sb.append(t)
        wo_sb = []
        for k in range(KHS):
            t = const.tile([PT, CH], bf16, tag=f"wo{k}", name=f"wo{k}")
            nc.sync.dma_start(t[:], d["wo"][k * PT:(k + 1) * PT, :])
            wo_sb.append(t)
        bo_sb = []
        for m in range(CH // PT):
            t = const.tile([PT, 1], f32, tag=f"bo{m}", name=f"bo{m}")
            nc.sync.dma_start(t[:], d["bo"][m * PT:(m + 1) * PT, :])
            bo_sb.append(t)
        eye_sb = const.tile([PT, PT], bf16, tag="eye", name="eye")
        nc.sync.dma_start(eye_sb[:], d["eye"][:, :])
        eye32_sb = const.tile([PT, PT], f32, tag="eye32", name="eye32")
        nc.sync.dma_start(eye32_sb[:], d["eye32"][:, :])

        for n in range(1, N):
            for k in range(KCH):
                t = cpool.tile([PT, T], bf16, tag=f"c{n}_{k}", name=f"c{n}_{k}")
                nc.sync.dma_start(t[:], d["c"][n, k * PT:(k + 1) * PT, :])
                c_sb[n, k] = t

        D_ps = psD.tile([PT, TT * HEADS * N], f32, name="D")
        Dv = D_ps[:].rearrange("p (a e n) -> p a e n", a=TT, n=N)
        # s_all[p, (tt, n)] = rsqrt(mean_n(c^2) + eps); sq_all holds sqrt
        sq_all = sp.tile([PT, TT * N], f32, tag="sq", name="sq_all")
        sqv = sq_all[:].rearrange("p (a n) -> p a n", n=N)
        s_all = sp.tile([PT, TT * N], f32, tag="s", name="s_all")
        sv = s_all[:].rearrange("p (a n) -> p a n", n=N)

        # ---- pass 0: per context slice n: squares, mean, s, dots ----
        for n in range(N):
            for tt in range(TT):
                for k in range(KCH):
                    nc.tensor.matmul(
                        Dv[:, tt, :, n],
                        c_sb[n, k][:, tt * PT:(tt + 1) * PT],
                        wq_sb[k][:],
                        start=(k == 0), stop=(k == KCH - 1),
                    )
            c2 = [c2p.tile([PT, T], bf16, tag="c2", name=f"c2_{n}_{_k}") for _k in range(KCH)]
            for k in range(KCH):
                if n % 2 == 0:
                    nc.scalar.activation(c2[k][:], c_sb[n, k][:], AF.Square)
                else:
                    nc.vector.tensor_mul(c2[k][:], c_sb[n, k][:],
                                         c_sb[n, k][:])
            mean_ps = psV.tile([PT, TT], f32, tag="v", name=f"mean{n}")
            for tt in range(TT):
                for k in range(KCH):
                    nc.tensor.matmul(
                        mean_ps[:, tt:tt + 1],
                        c2[k][:, tt * PT:(tt + 1) * PT],
                        invc_sb[k][:],
                        start=(k == 0), stop=(k == KCH - 1),
                    )
            nc.scalar.activation(sqv[:, :, n], mean_ps[:], AF.Sqrt,
                                 bias=eps_sb[:])
        nc.vector.reciprocal(s_all[:], sq_all[:])

        # ---- pass 1: per token tile: softmax, v matmul, h, transpose, out ----
        ht_sb = None
        for tt in range(TT):
            if tt % GRP == 0:
                ht_sb = [htp.tile([PT, GRP * PT], bf16, tag=f"ht{k}", name=f"ht{k}_{tt}")
                         for k in range(KHS)]
            s_bc = sv[:, tt:tt + 1, :].broadcast_to([PT, HEADS, N])
            # Dsc = dots * s (k-side rms scale), then one exp for all (e, n)
            Dsc = ep.tile([PT, HEADS * N], f32, tag="Dsc", name=f"Dsc{tt}")
            nc.vector.tensor_mul(
                Dsc[:].rearrange("p (e n) -> p e n", n=N),
                Dv[:, tt, :, :], s_bc)
            E = ep.tile([PT, HEADS * N], f32, tag="E", name=f"E{tt}")
            Ev = E[:].rearrange("p (e n) -> p e n", n=N)
            nc.scalar.activation(E[:], Dsc[:], AF.Exp)
            Z = ep.tile([PT, HEADS], f32, tag="Z", name=f"Z{tt}")
            nc.vector.tensor_reduce(Z[:], Ev, axis=AX.X, op=ALU.add)
            rZ = ep.tile([PT, HEADS], f32, tag="rZ", name=f"rZ{tt}")
            nc.vector.reciprocal(rZ[:], Z[:])
            # attnv[p, e, n] = E * (1/Z) [bcast over n] * s [bcast over e]
            rZ_bc = rZ[:].rearrange("p (e one) -> p e one", one=1) \
                         .broadcast_to([PT, HEADS, N])
            av_all = avp.tile([PT, HEADS * N], f32, tag="av", name=f"av{tt}")
            avv = av_all[:].rearrange("p (e n) -> p e n", n=N)
            nc.vector.tensor_mul(avv, Ev, rZ_bc)
            nc.vector.tensor_mul(avv, avv, s_bc)

            h = hp.tile([PT, HS], bf16, tag="h", name=f"h{tt}")
            for n in range(N):
                v_ps = psV.tile([PT, HS], f32, tag="v", name=f"v{tt}_{n}")
                for k in range(KCH):
                    nc.tensor.matmul(
                        v_ps[:],
                        c_sb[n, k][:, tt * PT:(tt + 1) * PT],
                        wv_sb[k][:],
                        start=(k == 0), stop=(k == KCH - 1),
                    )
                av_b = avv[:, :, n:n + 1].broadcast_to([PT, HEADS, HD])
                tgt = h if n == 0 else prodp.tile([PT, HS], bf16, tag="prod", name=f"prod{tt}_{n}")
                nc.vector.tensor_mul(
                    tgt[:].rearrange("p (e d) -> p e d", d=HD),
                    v_ps[:].rearrange("p (e d) -> p e d", d=HD),
                    av_b,
                )
                if n > 0:
                    eng = nc.gpsimd if n % 2 == 0 else nc.vector
                    eng.tensor_add(h[:], h[:], tgt[:])

            for m in range(KHS):
                tr = psT.tile([PT, PT], bf16, tag="tr", name=f"tr{tt}_{m}")
                nc.tensor.transpose(tr[:], h[:, m * PT:(m + 1) * PT], eye_sb[:])
                nc.scalar.copy(
                    ht_sb[m][:, (tt % GRP) * PT:(tt % GRP + 1) * PT], tr[:])

            if tt % GRP == GRP - 1:
                g = tt // GRP
                for m2 in range(CH // PT):
                    o_ps = psO.tile([PT, GRP * PT], f32, tag="o", name=f"ops{tt}_{m2}")
                    for k in range(KHS):
                        nc.tensor.matmul(
                            o_ps[:],
                            wo_sb[k][:, m2 * PT:(m2 + 1) * PT],
                            ht_sb[k][:],
                            start=(k == 0), stop=(k == KHS - 1),
                        )
                    o_sb = outp.tile([PT, GRP * PT], f32, tag="o", name=f"osb{tt}_{m2}")
                    nc.scalar.activation(o_sb[:], o_ps[:], AF.Identity,
                                         bias=bo_sb[m2][:])
                    nc.sync.dma_start(
                        d["out"][m2 * PT:(m2 + 1) * PT,
                                 g * GRP * PT:(g + 1) * GRP * PT],
                        o_sb[:])


def _build_nc():
    import concourse.tile as tile
    from concourse import bacc, mybir

    f32 = mybir.dt.float32
    bf16 = mybir.dt.bfloat16
    nc = bacc.Bacc(
        "TRN2",
        target_bir_lowering=False,
        debug=False,
        enable_asserts=False,
        num_devices=NCORES,
    )
    d = {
        "c": nc.dram_tensor("c", [N, CH, T], bf16, kind="ExternalInput").ap(),
        "wv": nc.dram_tensor("wv", [CH, HS], bf16, kind="ExternalInput").ap(),
        "wq": nc.dram_tensor("wq", [CH, HEADS], bf16, kind="ExternalInput").ap(),
        "wo": nc.dram_tensor("wo", [HS, CH], bf16, kind="ExternalInput").ap(),
        "bo": nc.dram_tensor("bo", [CH, 1], f32, kind="ExternalInput").ap(),
        "invc": nc.dram_tensor("invc", [CH, 1], bf16,
                               kind="ExternalInput").ap(),
        "eye": nc.dram_tensor("eye", [PT, PT], bf16, kind="ExternalInput").ap(),
        "eye32": nc.dram_tensor("eye32", [PT, PT], f32,
                                kind="ExternalInput").ap(),
        "out": nc.dram_tensor("out", [CH, T], f32, kind="ExternalOutput").ap(),
    }
    with tile.TileContext(nc) as tc:
        _kernel_body(nc, tc, d)
    nc.compile()
    return nc


_NC_CACHE = None


def _get_nc():
    global _NC_CACHE
    if _NC_CACHE is None:
        _NC_CACHE = _build_nc()
    return _NC_CACHE


def _make_in_maps(q, c, rms_w, emb, w1, b1, w2, b2, w_kv, w_out, b_out):
    q = np.asarray(q).astype(np.int64)
    c = np.asarray(c, dtype=np.float32)
    rms_w = np.asarray(rms_w, dtype=np.float32)
    emb = np.asarray(emb, dtype=np.float32)
    w1 = np.asarray(w1, dtype=np.float32)
    b1 = np.asarray(b1, dtype=np.float32)
    w2 = np.asarray(w2, dtype=np.float32)
    b2 = np.asarray(b2, dtype=np.float32)
    w_kv = np.asarray(w_kv, dtype=np.float32)
    w_out = np.asarray(w_out, dtype=np.float32)
    b_out = np.asarray(b_out, dtype=np.float32)

    # query path (tiny: 8 vectors), exact fp32 math as the reference
    qe = emb[q]                                   # [B, CH]
    x1 = qe @ w1 + b1
    h1 = x1 * (1.0 / (1.0 + np.exp(-x1)))         # silu
    qh = (h1 @ w2 + b2).reshape(B, HEADS, HD)

    wkv3 = w_kv.reshape(CH, HEADS, 2 * HD)
    w_k = wkv3[:, :, :HD]                         # [CH, HEADS, HD]
    w_v = wkv3[:, :, HD:]
    wv = np.ascontiguousarray(
        (rms_w[:, None, None] * w_v).reshape(CH, HS), dtype=np.float32)
    scale = float(HD) ** -0.5
    # wq[b, ch, e] = rms_w[ch] * scale * sum_d w_k[ch, e, d] * qh[b, e, d]
    wq_all = np.einsum("ced,bed->bce", w_k, qh).astype(np.float32)
    wq_all = wq_all * (scale * rms_w[None, :, None])

    import ml_dtypes
    bf = ml_dtypes.bfloat16
    shared = {
        "wv": wv.astype(bf),
        "wo": np.ascontiguousarray(w_out).astype(bf),
        "bo": np.ascontiguousarray(b_out.reshape(CH, 1), dtype=np.float32),
        "invc": np.full((CH, 1), 1.0 / CH, dtype=np.float32).astype(bf),
        "eye": np.eye(PT, dtype=np.float32).astype(bf),
        "eye32": np.eye(PT, dtype=np.float32),
    }
    in_maps = []
    for b in range(B):
        m = dict(shared)
        m["c"] = np.ascontiguousarray(c[b].reshape(N, CH, T)).astype(bf)
        m["wq"] = np.ascontiguousarray(wq_all[b]).astype(bf)
        in_maps.append(m)
    return in_maps


def _run(in_maps, **kwargs):
    from concourse import bass_utils

    nc = _get_nc()
    return bass_utils.run_bass_kernel_spmd(
        nc, in_maps, core_ids=list(range(NCORES)), **kwargs)


def kernel(q, c, rms_w, emb, w1, b1, w2, b2, w_kv, w_out, b_out):
    in_maps = _make_in_maps(q, c, rms_w, emb, w1, b1, w2, b2, w_kv, w_out,
                            b_out)
    res = _run(in_maps)
    outs = [np.asarray(res.results[b]["out"]).reshape(CH, H, W)
            for b in range(B)]
    return np.stack(outs, axis=0)
